# revision 26
# baseline (speedup 1.0000x reference)
"""Multi-head self-attention Bass kernel for Trainium2, 8 NeuronCores.

Strategy: data-parallel over batch (16 batches -> 2 per core), no collectives.
Per core, for each local batch:
  - X^T (d, n) layout prepared on host (transpose is free host work).
  - Q^T/K^T projections: lhsT = W chunks (natural layout), rhs = X^T  (f32r)
  - V projected directly in natural (g, v) layout with an appended ones
    column per head (for softmax denominators).
  - Scores computed transposed: S^T[g, q] per 128-g chunk (f32r, scale folded
    into W_query on host). Max-subtraction skipped: logits are bounded (~+-20)
    so exp() is safe in fp32/bf16 range.
  - exp on ACT (PSUM -> SBUF bf16), mask applied as a post-exp multiply by
    keep^T (host-transposed (1-mask) in bf16) -- exactly equivalent to the
    reference's -1e30 additive masking since exp(-1e30) == 0.
  - AV matmul with lhsT = [V_h | ones] (M=65): row 64 accumulates the softmax
    denominator d[q] for free.
  - Normalize via DVE reciprocal + GPSIMD partition-broadcast + multiply.
  - Output projection contracts (h,v)=512; result returned transposed
    (e, n) and fixed up on host.
"""

import numpy as np
import ml_dtypes

B, N, D, H, KD = 16, 1024, 512, 8, 64
NCORES = 8
B_LOC = B // NCORES  # 2
P = 128

_NC_CACHE = {}


def build_attention_nc(b_loc=B_LOC, n=N, repeat=1, hw_loop=0, skip=frozenset(), pairs_limit=None, s_tilepos=True, av_full=False, pipeline_av=False, u_bufs=17, xt_bufs=2, uraw_bufs=3, mask_split=False, av_banks=4):
    import concourse.bass as bass
    import concourse.mybir as mybir
    import concourse.tile as tile
    from concourse import bacc
    from contextlib import ExitStack

    F32 = mybir.dt.float32
    F32R = mybir.dt.float32r
    BF16 = mybir.dt.bfloat16
    EXP = mybir.ActivationFunctionType.Exp

    d = D
    n_gchunks = n // P          # 128-row key chunks
    n_dchunks = d // P          # contraction chunks for projections
    n_qhalves = n // 512        # 512-wide q slices (PSUM bank per matmul)
    n_pairs = H // 2

    nc = bacc.Bacc(trn_type="TRN2", target_bir_lowering=False, debug=False)

    qT_d = nc.dram_tensor("qT", [b_loc, d, n], F32R, kind="ExternalInput").ap()
    mask_d = nc.dram_tensor("maskT", [b_loc, n, n], BF16, kind="ExternalInput").ap()
    wq_d = nc.dram_tensor("wq", [d, d], F32R, kind="ExternalInput").ap()
    wk_d = nc.dram_tensor("wk", [d, d], F32R, kind="ExternalInput").ap()
    wv_d = nc.dram_tensor("wv", [d, d], F32R, kind="ExternalInput").ap()
    wo_d = nc.dram_tensor("wo", [d, d], F32R, kind="ExternalInput").ap()
    outT_d = nc.dram_tensor("outT", [b_loc, d, n], F32, kind="ExternalOutput").ap()

    with tile.TileContext(nc) as tc, ExitStack() as ctx, \
            nc.allow_low_precision(reason="bf16 attention weights by design"):
        # ---- pools ----
        const = ctx.enter_context(tc.tile_pool(name="const", bufs=1))
        xt_pool = ctx.enter_context(tc.tile_pool(name="xt", bufs=xt_bufs))
        keep_pool = ctx.enter_context(tc.tile_pool(name="keep", bufs=1))
        qt_pool = ctx.enter_context(tc.tile_pool(name="qt", bufs=4))
        kt_pool = ctx.enter_context(tc.tile_pool(name="kt", bufs=4))
        vones_pool = ctx.enter_context(tc.tile_pool(name="vones", bufs=1))
        u_pool = ctx.enter_context(tc.tile_pool(name="u", bufs=u_bufs))
        uraw_pool = ctx.enter_context(tc.tile_pool(name="uraw", bufs=uraw_bufs))
        heads_pool = ctx.enter_context(tc.tile_pool(name="heads", bufs=4))
        outsb_pool = ctx.enter_context(tc.tile_pool(name="outsb", bufs=2))
        r_pool = ctx.enter_context(tc.tile_pool(name="r", bufs=2))

        ps_s = ctx.enter_context(tc.tile_pool(name="ps_s", bufs=(2 if av_full else (4 - av_banks // 2)), space="PSUM"))
        ps_av = ctx.enter_context(tc.tile_pool(name="ps_av", bufs=av_banks, space="PSUM"))

        # ---- constants: weights + ones column ----
        wq_sb = const.tile([P, n_dchunks, d], F32R, tag="wq")
        wk_sb = const.tile([P, n_dchunks, d], F32R, tag="wk")
        wv_sb = const.tile([P, n_dchunks, d], F32R, tag="wv")
        wo_sb = const.tile([P, n_dchunks, d], F32R, tag="wo")
        for kc in range(n_dchunks):
            nc.gpsimd.dma_start(wq_sb[:, kc, :], wq_d[kc * P:(kc + 1) * P, :])
            nc.gpsimd.dma_start(wk_sb[:, kc, :], wk_d[kc * P:(kc + 1) * P, :])
            nc.gpsimd.dma_start(wv_sb[:, kc, :], wv_d[kc * P:(kc + 1) * P, :])
            nc.gpsimd.dma_start(wo_sb[:, kc, :], wo_d[kc * P:(kc + 1) * P, :])

        import contextlib
        loop_ctx = tc.For_i(0, hw_loop, 1) if hw_loop else contextlib.nullcontext()
        with loop_ctx:
          for b in [bb % b_loc for bb in range(repeat * b_loc)]:
            # ---- load X^T and keep^T ----
            xt = xt_pool.tile([P, n_dchunks, n], F32R)
            for kc in range(n_dchunks):
                nc.gpsimd.dma_start(xt[:, kc, :], qT_d[b, kc * P:(kc + 1) * P, :])
            keep = keep_pool.tile([P, n_gchunks, n], BF16, name="maskt")
            for g in range(n_gchunks):
                nc.gpsimd.dma_start(keep[:, g, :], mask_d[b, g * P:(g + 1) * P, :])

            # ---- Q^T / K^T projections (per head-pair) ----
            qt_tiles, kt_tiles = [], []
            if "proj" in skip:
                for dst_pool in (qt_pool, kt_pool):
                    t = dst_pool.tile([P, n], F32, tag="pf", name="pf")
                    nc.gpsimd.memset(t[:], 0.001)
                    tr = dst_pool.tile([P, n], F32R, tag="pfr", name="pfr")
                    nc.vector.tensor_copy(tr[:], t[:])
                    for _ in range(n_pairs):
                        (qt_tiles if dst_pool is qt_pool else kt_tiles).append(tr)
            for (w_sb, dst_list, dst_pool) in (() if "proj" in skip else (
                (wq_sb, qt_tiles, qt_pool),
                (wk_sb, kt_tiles, kt_pool),
            )):
                for p in range(n_pairs):
                    ps = ps_s.tile([P, n], F32, tag="s")
                    for kc in range(n_dchunks):
                        lhsT = w_sb[:, kc, p * P:(p + 1) * P]
                        for qh in range(n_qhalves):
                            nc.tensor.matmul(
                                ps[:, qh * 512:(qh + 1) * 512],
                                lhsT,
                                xt[:, kc, qh * 512:(qh + 1) * 512],
                                start=(kc == 0),
                                stop=(kc == n_dchunks - 1),
                            )
                    sb = dst_pool.tile([P, n], F32R)
                    nc.vector.tensor_copy(sb[:], ps[:])
                    dst_list.append(sb)

            # ---- V in natural (g, v) layout with ones columns ----
            vones = vones_pool.tile([P, n_gchunks, H * (KD + 1)], BF16)
            vones_h = vones[:].rearrange("p g (h x) -> p g h x", x=KD + 1)
            nc.gpsimd.memset(vones_h[:, :, :, KD:KD + 1], 1.0)
            for g in range(n_gchunks):
                if "proj" in skip:
                    break
                ps = ps_s.tile([P, n], F32, tag="s")
                for kc in range(n_dchunks):
                    nc.tensor.matmul(
                        ps[:, 0:d],
                        xt[:, kc, g * P:(g + 1) * P],
                        wv_sb[:, kc, :],
                        start=(kc == 0),
                        stop=(kc == n_dchunks - 1),
                    )
                nc.vector.tensor_copy(
                    vones_h[:, g, :, 0:KD],
                    ps[:, 0:d].rearrange("p (h x) -> p h x", x=KD),
                )

            # ---- attention per head-pair ----
            heads_tiles = [heads_pool.tile([P, n], F32R, tag="heads",
                                           name="heads")
                           for i in range(n_dchunks)]
            if "attn" in skip or "av" in skip or pairs_limit is not None:
                hf = heads_pool.tile([P, n], F32, tag="headsf", name="headsf")
                nc.gpsimd.memset(hf[:], 0.001)
                for htl in heads_tiles:
                    nc.vector.tensor_copy(htl[:], hf[:])
            def emit_av_chain(p, hh, qh, u_tiles_p):
                """One AV accumulation chain + normalization for head
                h = 2p+hh, q-half qh."""
                h = 2 * p + hh
                hv0 = h * KD
                av = ps_av.tile([KD + 1, 512], F32, tag="av", name="av")
                for g in range(n_gchunks):
                    nc.tensor.matmul(
                        av[:],
                        vones_tiles[p % 2][:, g, h * (KD + 1):(h + 1) * (KD + 1)],
                        u_tiles_p[(hh, g)][:, qh * 512:(qh + 1) * 512],
                        start=(g == 0),
                        stop=(g == n_gchunks - 1),
                    )
                r = r_pool.tile([1, 512], F32, tag="r", name="r")
                nc.vector.reciprocal(r[:], av[KD:KD + 1, :])
                rbc_sb = r_pool.tile([KD, 512], F32, tag="rbcsb", name="rbcsb")
                nc.gpsimd.partition_broadcast(rbc_sb[:], r[:])
                ht = heads_tiles[hv0 // P]
                nc.vector.tensor_mul(
                    ht[hv0 % P:hv0 % P + KD, qh * 512:(qh + 1) * 512],
                    av[0:KD, :],
                    rbc_sb[:],
                )

            vones_tiles = {0: vones, 1: vones}
            n_pairs_eff = pairs_limit if pairs_limit is not None else n_pairs
            prev = None  # (p, u_tiles) awaiting AV emission
            for p in range(n_pairs_eff):
                if "attn" in skip:
                    break
                u_tiles = {}
                av_slots = []
                if prev is not None and not pipeline_av:
                    pp, put = prev
                    for hh2 in range(2):
                        for qh2 in range(n_qhalves):
                            emit_av_chain(pp, hh2, qh2, put)
                    prev = None
                if prev is not None:
                    pp, put = prev
                    av_slots = [(pp, hh2, qh2, put)
                                for hh2 in range(2)
                                for qh2 in range(n_qhalves)]
                for g in range(n_gchunks):
                    for hh in range(2):
                        h = 2 * p + hh
                        rows = slice(hh * KD, (hh + 1) * KD)
                        if "s" not in skip:
                            ps = ps_s.tile([P, n], F32, tag="s")
                            for qh in range(n_qhalves):
                                qs = slice(qh * 512, (qh + 1) * 512)
                                nc.tensor.matmul(
                                    ps[:, qs],
                                    kt_tiles[p][rows, g * P:(g + 1) * P],
                                    qt_tiles[p][rows, qs],
                                    start=True,
                                    stop=True,
                                    tile_position=((hh * KD, 0) if s_tilepos
                                                   else None),
                                )
                        if "exp" in skip:
                            u = u_pool.tile([P, n], BF16, tag="u")
                            nc.gpsimd.memset(u[:], 0.001)
                        elif "mask" in skip:
                            u = u_pool.tile([P, n], BF16, tag="u")
                            nc.scalar.activation(u[:], ps[:], EXP)
                        else:
                            uraw = uraw_pool.tile([P, n], BF16, tag="uraw")
                            nc.scalar.activation(uraw[:], ps[:], EXP)
                            u = u_pool.tile([P, n], BF16, tag="u")
                            eng = (nc.gpsimd if (mask_split and g % 2 == 1)
                                   else nc.vector)
                            eng.tensor_mul(u[:], uraw[:], keep[:, g, :])
                        u_tiles[(hh, g)] = u
                    # interleave one previous-pair AV chain every other chunk
                    if av_slots and g % 2 == 1:
                        emit_av_chain(*av_slots.pop(0))
                for args in av_slots:
                    emit_av_chain(*args)
                if "av" in skip:
                    prev = None
                elif pipeline_av:
                    prev = (p, u_tiles)
                else:
                    prev = (p, u_tiles)
            if prev is not None and "attn" not in skip and "av" not in skip:
                pp, put = prev
                for hh2 in range(2):
                    for qh2 in range(n_qhalves):
                        emit_av_chain(pp, hh2, qh2, put)
            # ---- output projection: out^T[e, n] ----
            for eb in range(n_dchunks):
                if "oproj" in skip:
                    osb = outsb_pool.tile([P, n], F32, tag="osb")
                    nc.vector.tensor_copy(osb[:], keep[:, 0, :])
                    nc.gpsimd.dma_start(outT_d[b, eb * P:(eb + 1) * P, :], osb[:])
                    continue
                ps = ps_s.tile([P, n], F32, tag="s")
                for kc in range(n_dchunks):
                    lhsT = wo_sb[:, kc, eb * P:(eb + 1) * P]
                    for qh in range(n_qhalves):
                        nc.tensor.matmul(
                            ps[:, qh * 512:(qh + 1) * 512],
                            lhsT,
                            heads_tiles[kc][:, qh * 512:(qh + 1) * 512],
                            start=(kc == 0),
                            stop=(kc == n_dchunks - 1),
                        )
                osb = outsb_pool.tile([P, n], F32, tag="osb")
                nc.vector.tensor_copy(osb[:], ps[:])
                nc.gpsimd.dma_start(outT_d[b, eb * P:(eb + 1) * P, :], osb[:])

    nc.compile()
    return nc


def _get_nc(key=(B_LOC, N)):
    if key not in _NC_CACHE:
        _NC_CACHE[key] = build_attention_nc(*key)
    return _NC_CACHE[key]


def kernel(q, mask, W_query, W_key, W_val, W_out):
    from concourse.bass_utils import run_bass_kernel_spmd

    scale = np.float32(1.0 / np.sqrt(KD))
    qT = np.ascontiguousarray(q.transpose(0, 2, 1), dtype=np.float32)
    maskT = np.ascontiguousarray(
        (~mask).transpose(0, 2, 1)).astype(ml_dtypes.bfloat16)
    wq = np.ascontiguousarray(
        (W_query * scale).transpose(1, 0, 2).reshape(D, H * KD), dtype=np.float32)
    wk = np.ascontiguousarray(
        W_key.transpose(1, 0, 2).reshape(D, H * KD), dtype=np.float32)
    wv = np.ascontiguousarray(
        W_val.transpose(1, 0, 2).reshape(D, H * KD), dtype=np.float32)
    wo = np.ascontiguousarray(W_out.reshape(H * KD, D), dtype=np.float32)

    nc = _get_nc()
    in_maps = [
        {
            "qT": qT[c * B_LOC:(c + 1) * B_LOC],
            "maskT": maskT[c * B_LOC:(c + 1) * B_LOC],
            "wq": wq, "wk": wk, "wv": wv, "wo": wo,
        }
        for c in range(NCORES)
    ]
    last_exc = None
    for attempt in range(3):
        try:
            res = run_bass_kernel_spmd(nc, in_maps, core_ids=list(range(NCORES)))
            break
        except Exception as e:  # transient NRT device wedge -> retry
            last_exc = e
            import time as _time
            _time.sleep(5 * (attempt + 1))
    else:
        raise last_exc
    outT = np.concatenate([r["outT"] for r in res.results], axis=0)  # (16, 512, 1024)
    return np.ascontiguousarray(outT.transpose(0, 2, 1), dtype=np.float32)
